# revision 1
# baseline (speedup 1.0000x reference)
"""Causal self-attention (B=2, T=2048, L=1024, H=16) on 8 TRN2 NeuronCores.

Sharding: tensor-parallel over heads (4 heads/core) x data-parallel over batch
(cores 0-3 -> batch 0, cores 4-7 -> batch 1). Each core computes its heads'
attention plus the partial output projection; the host sums the 4 partials
per batch.

Per-core layouts (all fp32, matmuls run as float32r):
  stage A: Q^T/K^T [256, 2048] (head dims on partitions), V [2048, 4*65]
           (ones-augmented per head for the softmax denominator)
  stage B: S^T chunks [128 k, <=512 q] -> exp (no max subtraction; scores are
           ~N(0, 0.25) so exp is safe) -> PV accumulation in PSUM. Denominator
           appears in PSUM row 64; normalized via reciprocal + K=1 broadcast
           matmul.
  stage C: out[t, n] = y^T.T @ W_proj_slice, DMA'd straight per t-tile.
"""

import sys

for _p in ("/opt/trn_rl_repo",):
    if _p not in sys.path:
        sys.path.insert(0, _p)

import numpy as np

import concourse.bass as bass
import concourse.mybir as mybir
import concourse.tile as tile

F32 = mybir.dt.float32
BF16 = mybir.dt.bfloat16
EXP = mybir.ActivationFunctionType.Exp

B, T, L = 2, 2048, 1024
H = 16
DH = 64                      # head dim
HPC = 4                      # heads per core
HG = HPC * DH                # 256 cols per core per q/k/v
N_CORES = 8
KC = T // 128                # 16 k-chunks
NQB = T // 512               # 4 q-blocks
SCALE = 1.0 / np.sqrt(np.float32(L))  # rsqrt(L) per reference


def build_nc():
    nc = bass.Bass("TRN2", target_bir_lowering=False, debug=False)

    xT = nc.dram_tensor("xT", [L, T], BF16, kind="ExternalInput").ap()
    wa = nc.dram_tensor("wa", [L, 3 * HG], BF16, kind="ExternalInput").ap()
    wp = nc.dram_tensor("wp", [HG, L], BF16, kind="ExternalInput").ap()
    tm = nc.dram_tensor("trimaskb", [128, 128], F32, kind="ExternalInput").ap()
    idn = nc.dram_tensor("ident", [128, 128], F32, kind="ExternalInput").ap()
    out = nc.dram_tensor("out", [T, L], F32, kind="ExternalOutput").ap()

    with tile.TileContext(nc) as tc:
        with (
            tc.tile_pool(name="consts", bufs=1) as consts,
            tc.tile_pool(name="xp", bufs=8) as xp,
            tc.tile_pool(name="wap", bufs=8) as wap,
            tc.tile_pool(name="wpp", bufs=2) as wpp,
            tc.tile_pool(name="qk", bufs=2) as qk,
            tc.tile_pool(name="vp", bufs=16) as vp,
            tc.tile_pool(name="ytp", bufs=2) as ytp,
            tc.tile_pool(name="ptp", bufs=8) as ptp,
            tc.tile_pool(name="recp", bufs=8) as recp,
            tc.tile_pool(name="bcp", bufs=6) as bcp,
            tc.tile_pool(name="outp", bufs=4) as outp,
            tc.tile_pool(name="psmm", bufs=2, space="PSUM") as psmm,
            tc.tile_pool(name="psc", bufs=2, space="PSUM") as psc,
            tc.tile_pool(name="pss", bufs=2, space="PSUM") as pss,
            tc.tile_pool(name="pso", bufs=2, space="PSUM") as pso,
        ):
            # ---- constants & input loads ----
            tm_sb = consts.tile([128, 128], BF16)
            nc.gpsimd.dma_start(out=tm_sb[:], in_=tm[:])
            id_sb = consts.tile([128, 128], BF16)
            nc.gpsimd.dma_start(out=id_sb[:], in_=idn[:])
            ones_sb = consts.tile([128, 128], BF16)
            nc.vector.memset(ones_sb[:], 1.0)

            xt_sb = []
            wa_sb = []
            for kc in range(8):
                xt = xp.tile([128, T], BF16, tag="xt")
                nc.sync.dma_start(out=xt[:], in_=xT[kc * 128:(kc + 1) * 128, :])
                xt_sb.append(xt)
                wat = wap.tile([128, 3 * HG], BF16, tag="wa")
                nc.sync.dma_start(out=wat[:], in_=wa[kc * 128:(kc + 1) * 128, :])
                wa_sb.append(wat)
            wp_sb = []
            for i in range(2):
                wpt = wpp.tile([128, L], BF16, tag="wp")
                nc.sync.dma_start(out=wpt[:], in_=wp[i * 128:(i + 1) * 128, :])
                wp_sb.append(wpt)

            # ---- stage A: Q^T, K^T [256, T]; V_aug [T, 4*65] ----
            qt = [qk.tile([128, T], BF16, tag="qt", name=f"qt{m}") for m in range(2)]
            kt = [qk.tile([128, T], BF16, tag="kt", name=f"kt{m}") for m in range(2)]
            for nb in range(NQB):
                for dst, coff in ((qt, 0), (kt, HG)):
                    for m in range(2):
                        ps = psmm.tile([128, 512], F32, tag="mm")
                        for kc in range(8):
                            nc.tensor.matmul(
                                ps[:],
                                wa_sb[kc][:, coff + m * 128:coff + (m + 1) * 128],
                                xt_sb[kc][:, nb * 512:(nb + 1) * 512],
                                start=(kc == 0),
                                stop=(kc == 7),
                            )
                        nc.vector.tensor_copy(dst[m][:, nb * 512:(nb + 1) * 512], ps[:])

            va_sb = []
            for tt in range(KC):
                ps = psmm.tile([128, 512], F32, tag="mm")
                for kc in range(8):
                    nc.tensor.matmul(
                        ps[:, 0:HG],
                        xt_sb[kc][:, tt * 128:(tt + 1) * 128],
                        wa_sb[kc][:, 2 * HG:3 * HG],
                        start=(kc == 0),
                        stop=(kc == 7),
                    )
                va = vp.tile([128, HPC * 65], BF16, tag="va")
                nc.vector.tensor_copy(
                    va.rearrange("p (h c) -> p h c", c=65)[:, :, 0:64],
                    ps[:, 0:HG].rearrange("p (h c) -> p h c", c=64)[:, :, :],
                )
                nc.vector.memset(va.rearrange("p (h c) -> p h c", c=65)[:, :, 64:65], 1.0)
                va_sb.append(va)

            # ---- stage B + C interleaved per q-block ----
            yt = [ytp.tile([128, T], BF16, tag="yt", name=f"yt{m}") for m in range(2)]
            for qb in range(NQB):
                for pr in range(2):
                    po = {}
                    for hh in range(2):
                        po[hh] = pso.tile([65, 512], F32, tag="po", name=f"po{hh}")
                    nkc = 4 * qb + 4
                    for kc in range(nkc):
                        j = kc - 4 * qb
                        for hh in range(2):
                            h = 2 * pr + hh
                            hb = hh * 64
                            if j < 0:
                                q0, ncols = qb * 512, 512
                            else:
                                q0, ncols = qb * 512 + 128 * j, 512 - 128 * j
                            ps_s = pss.tile([128, 512], F32, tag="pss")
                            nc.tensor.matmul(
                                ps_s[:, 0:ncols],
                                kt[pr][hb:hb + 64, kc * 128:(kc + 1) * 128],
                                qt[pr][hb:hb + 64, q0:q0 + ncols],
                                start=True,
                                stop=(j < 0),
                            )
                            if j >= 0:
                                # add -1e30 above the diagonal: psum += I.T @ tri_bias
                                nc.tensor.matmul(
                                    ps_s[:, 0:128],
                                    id_sb[:],
                                    tm_sb[:],
                                    start=False,
                                    stop=True,
                                )
                            pt = ptp.tile([128, 512], BF16, tag="pt")
                            nc.scalar.activation(pt[:, 0:ncols], ps_s[:, 0:ncols], EXP,
                                                 scale=float(SCALE))
                            a0 = q0 - qb * 512
                            nc.tensor.matmul(
                                po[hh][:, a0:512],
                                va_sb[kc][:, h * 65:(h + 1) * 65],
                                pt[:, 0:ncols],
                                start=(kc == 0),
                                stop=(kc == nkc - 1),
                                skip_group_check=(0 < kc < nkc - 1),
                            )
                    # normalize: yT = po[0:64] * broadcast(1/po[64])
                    bs = bcp.tile([128, 512], F32, tag="bc")
                    for hh in range(2):
                        rec = recp.tile([65, 512], BF16, tag="rec")
                        with nc.allow_low_precision(reason="softmax denom reciprocal to bf16 for matmul broadcast"):
                            nc.vector.reciprocal(rec[64:65, :], po[hh][64:65, :])
                        bp = psmm.tile([128, 512], F32, tag="mm")
                        nc.tensor.matmul(bp[:], ones_sb[64:65, :],
                                         rec[64:65, :], start=True, stop=True)
                        nc.vector.tensor_copy(bs[hh * 64:(hh + 1) * 64, :], bp[hh * 64:(hh + 1) * 64, :])
                    for hh in range(2):
                        nc.vector.tensor_mul(
                            yt[pr][hh * 64:(hh + 1) * 64, qb * 512:(qb + 1) * 512],
                            po[hh][0:64, :],
                            bs[hh * 64:(hh + 1) * 64, :],
                        )

                # ---- stage C for this q-block's 4 t-tiles ----
                for tt in range(4 * qb, 4 * qb + 4):
                    osb = outp.tile([128, L], F32, tag="osb")
                    for nn in range(2):
                        ps = psc.tile([128, 512], F32, tag="psc")
                        for pr in range(2):
                            nc.tensor.matmul(
                                ps[:],
                                yt[pr][:, tt * 128:(tt + 1) * 128],
                                wp_sb[pr][:, nn * 512:(nn + 1) * 512],
                                start=(pr == 0),
                                stop=(pr == 1),
                            )
                        nc.vector.tensor_copy(osb[:, nn * 512:(nn + 1) * 512], ps[:])
                    nc.sync.dma_start(out=out[tt * 128:(tt + 1) * 128, :], in_=osb[:])

    import os as _os
    if not _os.environ.get("KERNEL_SKIP_WAITFIX"):
        _fix_matmul_waits(nc)
    return nc


def _fix_matmul_waits(nc):
    """walrus caps sync-wait commands at one per hardware instruction.
    Tile can emit more. Two safe fixes, applied in order:
    1. drop waits on the instruction's own engine semaphore that are already
       guaranteed by in-order retirement of earlier same-stream instructions;
    2. for any instruction still holding >1 wait, insert same-engine NoOps
       immediately before it, each carrying one excess wait (the waits still
       all execute before the instruction dispatches).
    """
    import bass_rust
    import concourse.mybir as mybir

    SKIP = (mybir.InstEventSemaphore, mybir.InstCall,
            mybir.InstUnconditionalBranch)
    nop_id = [0]

    for f in nc.m.functions:
        for blk in f.blocks:
            insts = list(blk.instructions)
            eng_sem_incs = {}
            pos_incs = []
            sem_owner = {}
            async_sems = set()
            for inst in insts:
                eng = getattr(inst, "engine", None)
                si = inst.sync_info
                pos_incs.append(dict(eng_sem_incs.get(eng, {})))
                if si is not None and eng is not None:
                    is_async = isinstance(
                        inst, (mybir.InstDMACopy, mybir.InstCollectiveCompute))
                    d = eng_sem_incs.setdefault(eng, {})
                    for u in si.on_update:
                        if is_async:
                            # DMA/collective sems fire at transfer completion,
                            # not instruction retirement - never FIFO-safe
                            async_sems.add(u.id)
                        d[u.id] = d.get(u.id, 0) + u.update_value
                        sem_owner.setdefault(u.id, eng)
            out = []
            changed = False
            for i, inst in enumerate(insts):
                si = inst.sync_info
                eng = getattr(inst, "engine", None)
                if si is None or eng is None or isinstance(inst, SKIP):
                    out.append(inst)
                    continue
                waits = list(si.on_wait)
                kept = waits
                if len(kept) > 1:
                    for w in kept[:-1]:
                        nop = mybir.InstNoOp(name=f"I-waitnop-{nop_id[0]}")
                        nop_id[0] += 1
                        nop.engine = eng
                        nop.sync_info = bass_rust.SyncInfo(on_wait=[w], on_update=[])
                        out.append(nop)
                    kept = kept[-1:]
                if len(kept) != len(waits):
                    inst.sync_info = bass_rust.SyncInfo(
                        on_wait=kept, on_update=list(si.on_update))
                    changed = True
                out.append(inst)
            if changed or len(out) != len(insts):
                blk.instructions = out


def make_in_maps(x, W_attn, W_proj):
    x = np.ascontiguousarray(np.asarray(x, dtype=np.float32))
    W_attn = np.ascontiguousarray(np.asarray(W_attn, dtype=np.float32))
    W_proj = np.ascontiguousarray(np.asarray(W_proj, dtype=np.float32))
    # [k, q] layout: invalid where q < k gets -1e30 (becomes exp -> 0).
    # scale by 1/SCALE so the exp's scale multiplier cancels.
    trimaskb = np.where(np.triu(np.ones((128, 128), bool)), 0.0, -3e30).astype(np.float32)
    ident = np.eye(128, dtype=np.float32)
    in_maps = []
    for c in range(N_CORES):
        b, hg = c // 4, c % 4
        cs = slice(hg * HG, (hg + 1) * HG)
        wa = np.concatenate(
            [W_attn[:, 0 * L:][:, cs], W_attn[:, 1 * L:][:, cs], W_attn[:, 2 * L:][:, cs]],
            axis=1,
        )
        import ml_dtypes
        bf16 = ml_dtypes.bfloat16
        in_maps.append({
            "xT": np.ascontiguousarray(x[b].T.astype(bf16)),
            "wa": np.ascontiguousarray(wa.astype(bf16)),
            "wp": np.ascontiguousarray(W_proj[cs, :].astype(bf16)),
            "trimaskb": trimaskb,
            "ident": ident,
        })
    return in_maps


_NC_CACHE = None


def kernel(x, W_attn, W_proj, **run_kwargs):
    global _NC_CACHE
    from concourse.bass_utils import run_bass_kernel_spmd

    if _NC_CACHE is None:
        _NC_CACHE = build_nc()
    nc = _NC_CACHE
    in_maps = make_in_maps(x, W_attn, W_proj)
    res = run_bass_kernel_spmd(nc, in_maps, list(range(N_CORES)), **run_kwargs)
    results = res.results if hasattr(res, "results") else res
    out = np.zeros((B, T, L), np.float32)
    for c in range(N_CORES):
        out[c // 4] += results[c]["out"]
    if run_kwargs:
        kernel.last_results = res
    return out



# revision 4
# speedup vs baseline: 24.6927x; 24.6927x over previous
"""Causal self-attention (B=2, T=2048, L=1024, H=16) on 8 TRN2 NeuronCores.

Sharding: tensor-parallel over heads (4 heads/core) x data-parallel over batch
(cores 0-3 -> batch 0, cores 4-7 -> batch 1). Each core computes its heads'
attention plus the partial output projection; the host sums the 4 partials
per batch.

Per-core layouts (all fp32, matmuls run as float32r):
  stage A: Q^T/K^T [256, 2048] (head dims on partitions), V [2048, 4*65]
           (ones-augmented per head for the softmax denominator)
  stage B: S^T chunks [128 k, <=512 q] -> exp (no max subtraction; scores are
           ~N(0, 0.25) so exp is safe) -> PV accumulation in PSUM. Denominator
           appears in PSUM row 64; normalized via reciprocal + K=1 broadcast
           matmul.
  stage C: out[t, n] = y^T.T @ W_proj_slice, DMA'd straight per t-tile.
"""

import sys

for _p in ("/opt/trn_rl_repo",):
    if _p not in sys.path:
        sys.path.insert(0, _p)

import numpy as np

import concourse.bass as bass
import concourse.mybir as mybir
import concourse.tile as tile

F32 = mybir.dt.float32
BF16 = mybir.dt.bfloat16
EXP = mybir.ActivationFunctionType.Exp

B, T, L = 2, 2048, 1024
H = 16
DH = 64                      # head dim
HPC = 4                      # heads per core
HG = HPC * DH                # 256 cols per core per q/k/v
N_CORES = 8
KC = T // 128                # 16 k-chunks
NQB = T // 512               # 4 q-blocks
SCALE = 1.0 / np.sqrt(np.float32(L))  # rsqrt(L) per reference


def build_nc(iters=1):
    nc = bass.Bass("TRN2", target_bir_lowering=False, debug=False)

    xT = nc.dram_tensor("xT", [L, T], BF16, kind="ExternalInput").ap()
    wa = nc.dram_tensor("wa", [L, 3 * HG], BF16, kind="ExternalInput").ap()
    wp = nc.dram_tensor("wp", [HG, L], BF16, kind="ExternalInput").ap()
    tm = nc.dram_tensor("trimaskb", [128, 128], F32, kind="ExternalInput").ap()
    idn = nc.dram_tensor("ident", [128, 128], F32, kind="ExternalInput").ap()
    out = nc.dram_tensor("out", [T, L], F32, kind="ExternalOutput").ap()

    with tile.TileContext(nc) as tc:
        with (
            tc.tile_pool(name="consts", bufs=1) as consts,
            tc.tile_pool(name="xp", bufs=8) as xp,
            tc.tile_pool(name="wap", bufs=8) as wap,
            tc.tile_pool(name="wpp", bufs=2) as wpp,
            tc.tile_pool(name="qk", bufs=2) as qk,
            tc.tile_pool(name="vp", bufs=16) as vp,
            tc.tile_pool(name="ytp", bufs=2) as ytp,
            tc.tile_pool(name="ptp", bufs=8) as ptp,
            tc.tile_pool(name="recp", bufs=8) as recp,
            tc.tile_pool(name="bcp", bufs=6) as bcp,
            tc.tile_pool(name="outp", bufs=4) as outp,
            tc.tile_pool(name="psmm", bufs=2, space="PSUM") as psmm,
            tc.tile_pool(name="psc", bufs=2, space="PSUM") as psc,
            tc.tile_pool(name="pss", bufs=2, space="PSUM") as pss,
            tc.tile_pool(name="pso", bufs=2, space="PSUM") as pso,
        ):
            # ---- constants ----
            tm_sb = consts.tile([128, 128], BF16)
            nc.gpsimd.dma_start(out=tm_sb[:], in_=tm[:])
            id_sb = consts.tile([128, 128], BF16)
            nc.gpsimd.dma_start(out=id_sb[:], in_=idn[:])
            ones_sb = consts.tile([128, 128], BF16)
            nc.vector.memset(ones_sb[:], 1.0)

            for _it in range(iters):
                _body(nc, tc, xT, wa, wp, out, tm_sb, id_sb, ones_sb,
                      xp, wap, wpp, qk, vp, ytp, ptp, recp, bcp, outp,
                      psmm, psc, pss, pso, _it)

    import os as _os
    if not _os.environ.get("KERNEL_SKIP_WAITFIX"):
        _fix_matmul_waits(nc)
    return nc


def _body(nc, tc, xT, wa, wp, out, tm_sb, id_sb, ones_sb,
          xp, wap, wpp, qk, vp, ytp, ptp, recp, bcp, outp,
          psmm, psc, pss, pso, it):
            # ---- input loads ----
            xt_sb = []
            wa_sb = []
            for kc in range(8):
                xt = xp.tile([128, T], BF16, tag="xt")
                nc.sync.dma_start(out=xt[:], in_=xT[kc * 128:(kc + 1) * 128, :])
                xt_sb.append(xt)
                wat = wap.tile([128, 3 * HG], BF16, tag="wa")
                nc.sync.dma_start(out=wat[:], in_=wa[kc * 128:(kc + 1) * 128, :])
                wa_sb.append(wat)
            wp_sb = []
            for i in range(2):
                wpt = wpp.tile([128, L], BF16, tag="wp")
                nc.sync.dma_start(out=wpt[:], in_=wp[i * 128:(i + 1) * 128, :])
                wp_sb.append(wpt)

            # ---- stage A: Q^T, K^T [256, T]; V_aug [T, 4*65] ----
            qt = [qk.tile([128, T], BF16, tag="qt", name=f"qt{m}_{it}") for m in range(2)]
            kt = [qk.tile([128, T], BF16, tag="kt", name=f"kt{m}_{it}") for m in range(2)]
            for nb in range(NQB):
                for dst, coff in ((qt, 0), (kt, HG)):
                    for m in range(2):
                        ps = psmm.tile([128, 512], F32, tag="mm")
                        for kc in range(8):
                            nc.tensor.matmul(
                                ps[:],
                                wa_sb[kc][:, coff + m * 128:coff + (m + 1) * 128],
                                xt_sb[kc][:, nb * 512:(nb + 1) * 512],
                                start=(kc == 0),
                                stop=(kc == 7),
                            )
                        nc.vector.tensor_copy(dst[m][:, nb * 512:(nb + 1) * 512], ps[:])

            va_sb = []
            for tt in range(KC):
                ps = psmm.tile([128, 512], F32, tag="mm")
                for kc in range(8):
                    nc.tensor.matmul(
                        ps[:, 0:HG],
                        xt_sb[kc][:, tt * 128:(tt + 1) * 128],
                        wa_sb[kc][:, 2 * HG:3 * HG],
                        start=(kc == 0),
                        stop=(kc == 7),
                    )
                va = vp.tile([128, HPC * 65], BF16, tag="va")
                nc.vector.tensor_copy(
                    va.rearrange("p (h c) -> p h c", c=65)[:, :, 0:64],
                    ps[:, 0:HG].rearrange("p (h c) -> p h c", c=64)[:, :, :],
                )
                nc.vector.memset(va.rearrange("p (h c) -> p h c", c=65)[:, :, 64:65], 1.0)
                va_sb.append(va)

            # ---- stage B + C interleaved per q-block ----
            yt = [ytp.tile([128, T], BF16, tag="yt", name=f"yt{m}_{it}") for m in range(2)]
            for qb in range(NQB):
                for pr in range(2):
                    po = {}
                    for hh in range(2):
                        po[hh] = pso.tile([65, 512], F32, tag="po", name=f"po{hh}")
                    nkc = 4 * qb + 4
                    for kc in range(nkc):
                        j = kc - 4 * qb
                        for hh in range(2):
                            h = 2 * pr + hh
                            hb = hh * 64
                            if j < 0:
                                q0, ncols = qb * 512, 512
                            else:
                                q0, ncols = qb * 512 + 128 * j, 512 - 128 * j
                            ps_s = pss.tile([128, 512], F32, tag="pss")
                            nc.tensor.matmul(
                                ps_s[:, 0:ncols],
                                kt[pr][hb:hb + 64, kc * 128:(kc + 1) * 128],
                                qt[pr][hb:hb + 64, q0:q0 + ncols],
                                start=True,
                                stop=(j < 0),
                            )
                            if j >= 0:
                                # add -1e30 above the diagonal: psum += I.T @ tri_bias
                                nc.tensor.matmul(
                                    ps_s[:, 0:128],
                                    id_sb[:],
                                    tm_sb[:],
                                    start=False,
                                    stop=True,
                                )
                            pt = ptp.tile([128, 512], BF16, tag="pt")
                            nc.scalar.activation(pt[:, 0:ncols], ps_s[:, 0:ncols], EXP,
                                                 scale=float(SCALE))
                            a0 = q0 - qb * 512
                            nc.tensor.matmul(
                                po[hh][:, a0:512],
                                va_sb[kc][:, h * 65:(h + 1) * 65],
                                pt[:, 0:ncols],
                                start=(kc == 0),
                                stop=(kc == nkc - 1),
                                skip_group_check=(0 < kc < nkc - 1),
                            )
                    # normalize: yT = po[0:64] * broadcast(1/po[64])
                    bs = bcp.tile([128, 512], F32, tag="bc")
                    for hh in range(2):
                        rec = recp.tile([65, 512], BF16, tag="rec")
                        with nc.allow_low_precision(reason="softmax denom reciprocal to bf16 for matmul broadcast"):
                            nc.vector.reciprocal(rec[64:65, :], po[hh][64:65, :])
                        bp = psmm.tile([128, 512], F32, tag="mm")
                        nc.tensor.matmul(bp[:], ones_sb[64:65, :],
                                         rec[64:65, :], start=True, stop=True)
                        nc.vector.tensor_copy(bs[hh * 64:(hh + 1) * 64, :], bp[hh * 64:(hh + 1) * 64, :])
                    for hh in range(2):
                        nc.vector.tensor_mul(
                            yt[pr][hh * 64:(hh + 1) * 64, qb * 512:(qb + 1) * 512],
                            po[hh][0:64, :],
                            bs[hh * 64:(hh + 1) * 64, :],
                        )

                # ---- stage C for this q-block's 4 t-tiles ----
                for tt in range(4 * qb, 4 * qb + 4):
                    osb = outp.tile([128, L], F32, tag="osb")
                    for nn in range(2):
                        ps = psc.tile([128, 512], F32, tag="psc")
                        for pr in range(2):
                            nc.tensor.matmul(
                                ps[:],
                                yt[pr][:, tt * 128:(tt + 1) * 128],
                                wp_sb[pr][:, nn * 512:(nn + 1) * 512],
                                start=(pr == 0),
                                stop=(pr == 1),
                            )
                        nc.vector.tensor_copy(osb[:, nn * 512:(nn + 1) * 512], ps[:])
                    nc.sync.dma_start(out=out[tt * 128:(tt + 1) * 128, :], in_=osb[:])


def _fix_matmul_waits(nc):
    """walrus caps sync-wait commands at one per hardware instruction.
    Tile can emit more. Two safe fixes, applied in order:
    1. drop waits on the instruction's own engine semaphore that are already
       guaranteed by in-order retirement of earlier same-stream instructions;
    2. for any instruction still holding >1 wait, insert same-engine NoOps
       immediately before it, each carrying one excess wait (the waits still
       all execute before the instruction dispatches).
    """
    import bass_rust
    import concourse.mybir as mybir

    SKIP = (mybir.InstEventSemaphore, mybir.InstCall,
            mybir.InstUnconditionalBranch)
    nop_id = [0]

    for f in nc.m.functions:
        for blk in f.blocks:
            insts = list(blk.instructions)
            eng_sem_incs = {}
            pos_incs = []
            sem_owner = {}
            async_sems = set()
            for inst in insts:
                eng = getattr(inst, "engine", None)
                si = inst.sync_info
                pos_incs.append(dict(eng_sem_incs.get(eng, {})))
                if si is not None and eng is not None:
                    is_async = isinstance(
                        inst, (mybir.InstDMACopy, mybir.InstCollectiveCompute))
                    d = eng_sem_incs.setdefault(eng, {})
                    for u in si.on_update:
                        if is_async:
                            # DMA/collective sems fire at transfer completion,
                            # not instruction retirement - never FIFO-safe
                            async_sems.add(u.id)
                        d[u.id] = d.get(u.id, 0) + u.update_value
                        sem_owner.setdefault(u.id, eng)
            out = []
            changed = False
            for i, inst in enumerate(insts):
                si = inst.sync_info
                eng = getattr(inst, "engine", None)
                if si is None or eng is None or isinstance(inst, SKIP):
                    out.append(inst)
                    continue
                waits = list(si.on_wait)
                kept = waits
                if len(kept) > 1:
                    for w in kept[:-1]:
                        nop = mybir.InstNoOp(name=f"I-waitnop-{nop_id[0]}")
                        nop_id[0] += 1
                        nop.engine = eng
                        nop.sync_info = bass_rust.SyncInfo(on_wait=[w], on_update=[])
                        out.append(nop)
                    kept = kept[-1:]
                if len(kept) != len(waits):
                    inst.sync_info = bass_rust.SyncInfo(
                        on_wait=kept, on_update=list(si.on_update))
                    changed = True
                out.append(inst)
            if changed or len(out) != len(insts):
                blk.instructions = out


def make_in_maps(x, W_attn, W_proj):
    x = np.ascontiguousarray(np.asarray(x, dtype=np.float32))
    W_attn = np.ascontiguousarray(np.asarray(W_attn, dtype=np.float32))
    W_proj = np.ascontiguousarray(np.asarray(W_proj, dtype=np.float32))
    # [k, q] layout: invalid where q < k gets -1e30 (becomes exp -> 0).
    # scale by 1/SCALE so the exp's scale multiplier cancels.
    trimaskb = np.where(np.triu(np.ones((128, 128), bool)), 0.0, -3e30).astype(np.float32)
    ident = np.eye(128, dtype=np.float32)
    in_maps = []
    for c in range(N_CORES):
        b, hg = c // 4, c % 4
        cs = slice(hg * HG, (hg + 1) * HG)
        wa = np.concatenate(
            [W_attn[:, 0 * L:][:, cs], W_attn[:, 1 * L:][:, cs], W_attn[:, 2 * L:][:, cs]],
            axis=1,
        )
        import ml_dtypes
        bf16 = ml_dtypes.bfloat16
        in_maps.append({
            "xT": np.ascontiguousarray(x[b].T.astype(bf16)),
            "wa": np.ascontiguousarray(wa.astype(bf16)),
            "wp": np.ascontiguousarray(W_proj[cs, :].astype(bf16)),
            "trimaskb": trimaskb,
            "ident": ident,
        })
    return in_maps


_NC_CACHE = None


def kernel(x, W_attn, W_proj, **run_kwargs):
    global _NC_CACHE
    from concourse.bass_utils import run_bass_kernel_spmd

    if _NC_CACHE is None:
        _NC_CACHE = build_nc()
    nc = _NC_CACHE
    in_maps = make_in_maps(x, W_attn, W_proj)
    res = run_bass_kernel_spmd(nc, in_maps, list(range(N_CORES)), **run_kwargs)
    results = res.results if hasattr(res, "results") else res
    out = np.zeros((B, T, L), np.float32)
    for c in range(N_CORES):
        out[c // 4] += results[c]["out"]
    if run_kwargs:
        kernel.last_results = res
    return out

